# revision 1
# baseline (speedup 1.0000x reference)
"""BCH/RS systematic encoder kernel for Trainium2 (8 NeuronCores, data parallel).

Computes out = concat([msg, (msg @ Gp) mod 2], axis=-1) for
msg [16384, 1000] f32 of 0/1 bits and Gp [1000, 256] f32 of 0/1 bits.

Design (per core, 2048 rows, 16 chunks of 128):
  - SWDGE cast-load msg chunk f32 -> bf16 SBUF (0/1 exact in bf16)
  - SWDGE cast-store bf16 -> f32 to out[:, :1000] (systematic copy-through)
  - DMA xbar transpose (2-byte) 128x128 blocks: msg natural -> msgT [k, m]
  - 8 accumulating bf16 matmuls: psum[m,256] += msgT_k.T @ Gp_k (fp32 accum, exact)
  - DVE tensor_scalar mod 2.0 on psum -> SBUF f32
  - store parity to out[:, 1000:1256]
HBM traffic/core = 8.19 MB read + 10.29 MB write (the minimum).
"""

import os
import sys

import numpy as np

if os.path.isdir("/opt/trn_rl_repo") and "/opt/trn_rl_repo" not in sys.path:
    sys.path.insert(0, "/opt/trn_rl_repo")

import ml_dtypes

import concourse.bacc as bacc
import concourse.mybir as mybir
import concourse.tile as tile
from concourse.bass_utils import run_bass_kernel_spmd

BATCH = 16384
MSG = 1000
NPAR = 256
NCORES = 8
ROWS = BATCH // NCORES  # 2048
P = 128
KCH = 8  # k chunks; padded K = 1024
KPAD = KCH * P

# test.py pokes these for profiling
TRACE = False
LAST_RESULT = None

_CACHE = {}


def build_nc(rows=ROWS):
    """Emit the Bass/Tile IR for one core handling `rows` rows."""
    mch = rows // P
    nc = bacc.Bacc("TRN2", target_bir_lowering=False, debug=False)
    msg = nc.dram_tensor("msg", [rows, MSG], mybir.dt.float32, kind="ExternalInput")
    gp = nc.dram_tensor("gp", [P, KCH * NPAR], mybir.dt.bfloat16, kind="ExternalInput")
    out = nc.dram_tensor(
        "out", [rows, MSG + NPAR], mybir.dt.float32, kind="ExternalOutput"
    )

    SC = 2  # m-chunks per superchunk (SWDGE/DVE batching granularity)
    n_super = mch // SC
    LAG = 1  # stores trail compute by this many superchunks
    msg3 = msg[:, :].rearrange("(s c p) k -> s c p k", c=SC, p=P)
    out3 = out[:, :].rearrange("(s c p) k -> s c p k", c=SC, p=P)

    with tile.TileContext(nc) as tc:
        with (
            tc.tile_pool(name="gpool", bufs=1) as gpool,
            # every superchunk's a-tile is resident at once: loads all run
            # upfront and never wait on a recycled slot (or a store's SWDGE
            # semaphore lane)
            tc.tile_pool(name="apool", bufs=n_super + 1) as apool,
            tc.tile_pool(name="bpool", bufs=6) as bpool,
            tc.tile_pool(name="cpool", bufs=4) as cpool,
            tc.tile_pool(name="epool", bufs=4) as epool,
            tc.tile_pool(name="ppool", bufs=8, space="PSUM") as ppool,
        ):
            # Gp resident in SBUF: gsb[q, kb*256 + n] = Gp_padded[kb*128 + q, n]
            gsb = gpool.tile([P, KCH * NPAR], mybir.dt.bfloat16)
            nc.sync.dma_start(out=gsb[:, :], in_=gp[:, :])

            a_tiles = {}
            es = {}

            # row stride must keep every a[:, c, :] slice 32B-aligned for the
            # xbar transpose: 1264 bf16 = 2528 B = 79*32
            ROWP = 1264

            def emit_load(si):
                # full output row in bf16: cols 0:1000 msg, 1000:1256 parity.
                # No zero-pad memset: the last k-chunk matmul contracts K=104,
                # so the PE never reads the transposed garbage rows.
                a = apool.tile([P, SC, ROWP], mybir.dt.bfloat16, tag="a")
                nc.gpsimd.dma_start(
                    out=a[:, :, 0:MSG], in_=msg3[si, :, :, :].rearrange("c p k -> p c k")
                )
                a_tiles[si] = a

            def emit_compute(si):
                a = a_tiles[si]
                # per-chunk xbar transpose: b[q, c*KCH + kb, p] = a[p, c, kb*128+q]
                # all on ONE HWDGE ring: concurrent xbar transposes from two
                # rings corrupt each other (shared xbar; this Tile does not
                # cross-engine-serialize them)
                b = bpool.tile([P, SC * KCH, P], mybir.dt.bfloat16, tag="b")
                for c in range(SC):
                    nc.sync.dma_start(
                        out=b[:, c * KCH : (c + 1) * KCH, :],
                        in_=a[:, c, 0:KPAD],
                        transpose=True,
                    )
                # both chunks accumulate side by side in one PSUM bank
                acc = ppool.tile([P, SC * NPAR], mybir.dt.float32, tag="acc")
                for c in range(SC):
                    for kb in range(KCH):
                        kk = P if kb < KCH - 1 else MSG - (KCH - 1) * P  # 104 tail
                        nc.tensor.matmul(
                            acc[:, c * NPAR : (c + 1) * NPAR],
                            b[0:kk, c * KCH + kb, :],
                            gsb[0:kk, kb * NPAR : (kb + 1) * NPAR],
                            start=(kb == 0),
                            stop=(kb == KCH - 1),
                        )
                # exact-integer f32 -> i32 eviction in ONE op on idle ACT
                c_i32 = cpool.tile([P, SC, NPAR], mybir.dt.int32, tag="c")
                nc.scalar.copy(
                    c_i32[:, :, :].rearrange("p c n -> p (c n)"), acc[:, :]
                )
                # mod 2 == AND 1 (bitVec op cannot cast, keep i32)
                e = epool.tile([P, SC, NPAR], mybir.dt.int32, tag="e")
                nc.vector.tensor_scalar(
                    e[:, :, :], c_i32[:, :, :], 1, None, mybir.AluOpType.bitwise_and
                )
                # parity into the output-row tile (0/1 exact in bf16)
                nc.vector.tensor_copy(a[:, :, MSG : MSG + NPAR], e[:, :, :])

            def emit_store(si):
                # single cast-store of the full rows: [p, c, 1256] bf16 -> f32
                a = a_tiles.pop(si)
                nc.gpsimd.dma_start(
                    out=out3[si, :, :, :].rearrange("c p k -> p c k"),
                    in_=a[:, :, 0 : MSG + NPAR],
                )

            for it in range(n_super):
                emit_load(it)
            # zero the transpose pad columns once per (fresh) slot, batched on
            # DVE before the compute chain (keeps CoreSim's uninit checker
            # happy; PE never reads those rows thanks to the K=104 tail)
            for it in range(n_super):
                nc.vector.memset(a_tiles[it][:, :, MSG:KPAD], 0)
            for it in range(n_super + LAG):
                if it < n_super:
                    emit_compute(it)
                k = it - LAG
                if 0 <= k < n_super:
                    emit_store(k)

    nc.compile()
    return nc


def prep_gp(Gp):
    """Pad Gp to 1024 rows and swizzle to the [128, 8*256] bf16 SBUF layout."""
    gp = np.asarray(Gp, dtype=np.float32)
    gp_pad = np.zeros((KPAD, NPAR), dtype=np.float32)
    gp_pad[:MSG] = gp
    gsw = gp_pad.reshape(KCH, P, NPAR).transpose(1, 0, 2).reshape(P, KCH * NPAR)
    return np.ascontiguousarray(gsw).astype(ml_dtypes.bfloat16)


def kernel(message_bits, Gp):
    global LAST_RESULT
    msg = np.ascontiguousarray(np.asarray(message_bits, dtype=np.float32))
    assert msg.shape == (BATCH, MSG), msg.shape
    gsw = prep_gp(Gp)

    if "nc" not in _CACHE:
        _CACHE["nc"] = build_nc()
    nc = _CACHE["nc"]

    in_maps = [
        {"msg": msg[i * ROWS : (i + 1) * ROWS], "gp": gsw} for i in range(NCORES)
    ]
    res = run_bass_kernel_spmd(
        nc, in_maps, core_ids=list(range(NCORES)), trace=TRACE
    )
    LAST_RESULT = res
    return np.concatenate([r["out"] for r in res.results], axis=0)



# revision 2
# speedup vs baseline: 1.2566x; 1.2566x over previous
"""BCH/RS systematic encoder kernel for Trainium2 (8 NeuronCores, data parallel).

Computes out = concat([msg, (msg @ Gp) mod 2], axis=-1) for
msg [16384, 1000] f32 of 0/1 bits and Gp [1000, 256] f32 of 0/1 bits.

Design (per core, 2048 rows, 8 superchunks of 2x128):
  - HWDGE (sync) load msg chunk f32 into the ld/st row tile A[:, :, 0:1000]
  - ACT cast f32 -> fp8e4 (0/1 exact) into F; fp8 pairs live in 16-bit granules
  - HWDGE xbar transpose of the GRANULE tile (2-byte lanes): 4 blocks of
    [128,128] granules per chunk -> G[q, m] = (msg[m, 2K], msg[m, 2K+1]),
    K = 128*gb + q.  Half the xbar traffic of a bf16 transpose.
  - 8 accumulating fp8 matmuls per chunk read G with byte-strided APs
    (even/odd k split); the k-interleave is folded into the host-side Gp
    slab layout (permuting the contraction index is free).
  - ACT psum->i32, DVE AND 1 (mod 2), DVE i32->f32 into A[:, :, 1000:1256]
  - SWDGE (gpsimd) store of the full f32 rows [p, c, 1256]
HBM traffic/core = 8.19 MB read + 10.29 MB write (the minimum); xbar adds
only 2.1 MB each way on the shared SDMA fabric.
"""

import os
import sys

import numpy as np

if os.path.isdir("/opt/trn_rl_repo") and "/opt/trn_rl_repo" not in sys.path:
    sys.path.insert(0, "/opt/trn_rl_repo")

import ml_dtypes

import concourse.bacc as bacc
import concourse.mybir as mybir
import concourse.tile as tile
from concourse.bass_utils import run_bass_kernel_spmd

BATCH = 16384
MSG = 1000
NPAR = 256
NCORES = 8
ROWS = BATCH // NCORES  # 2048
P = 128
KPAD = 1024  # fp8 columns after pad
NGRAN = KPAD // 2  # 512 16-bit granules
GB = NGRAN // P  # 4 granule blocks per chunk
OUTW = MSG + NPAR  # 1256

# test.py pokes these for profiling
TRACE = False
LAST_RESULT = None

_CACHE = {}


def build_nc(rows=ROWS):
    """Emit the Bass/Tile IR for one core handling `rows` rows."""
    nc = bacc.Bacc("TRN2", target_bir_lowering=False, debug=False)
    msg = nc.dram_tensor("msg", [rows, MSG], mybir.dt.float32, kind="ExternalInput")
    gp = nc.dram_tensor("gp", [P, 2 * GB * NPAR], mybir.dt.uint8, kind="ExternalInput")
    out = nc.dram_tensor("out", [rows, OUTW], mybir.dt.float32, kind="ExternalOutput")

    SC = 2  # chunks per superchunk
    n_super = rows // (SC * P)
    msg3 = msg[:, :].rearrange("(s c p) k -> s c p k", c=SC, p=P)
    out3 = out[:, :].rearrange("(s c p) k -> s c p k", c=SC, p=P)

    with tile.TileContext(nc) as tc:
        with (
            tc.tile_pool(name="gpool", bufs=1) as gpool,
            tc.tile_pool(name="apool", bufs=min(n_super, 8)) as apool,
            tc.tile_pool(name="fpool", bufs=4) as fpool,
            tc.tile_pool(name="tpool", bufs=4) as tpool,
            tc.tile_pool(name="cpool", bufs=3) as cpool,
            tc.tile_pool(name="epool", bufs=3) as epool,
            tc.tile_pool(name="ppool", bufs=6, space="PSUM") as ppool,
        ):
            # Gp slabs resident in SBUF: row q of slab s=(gb,off) holds
            # Gp_pad[2*(128*gb+q)+off, :] as fp8 bytes
            gsb = gpool.tile([P, 2 * GB * NPAR], mybir.dt.uint8)
            nc.sync.dma_start(out=gsb[:, :], in_=gp[:, :])
            gsb8 = gsb[:, :].bitcast(mybir.dt.float8e4)

            a_tiles = {}

            def emit_load(si):
                # full output row in f32: cols 0:1000 msg, 1000:1256 parity
                a = apool.tile([P, SC, OUTW], mybir.dt.float32, tag="a")
                nc.sync.dma_start(
                    out=a[:, :, 0:MSG], in_=msg3[si, :, :, :].rearrange("c p k -> p c k")
                )
                a_tiles[si] = a

            def emit_compute(si):
                a = a_tiles[si]
                # fp8 cast target, allocated as bf16 so the xbar sees 2-byte
                # granules; each granule = (fp8 k=2j, fp8 k=2j+1)
                f = fpool.tile([P, SC, NGRAN], mybir.dt.bfloat16, tag="f")
                f8 = f[:, :, :].bitcast(mybir.dt.float8e4)  # [P, SC, KPAD]
                # zero the pad so pad-row garbage can't turn into NaN*0 in PSUM
                nc.vector.memset(f[:, :, MSG // 2 :], 0)
                nc.scalar.copy(f8[:, :, 0:MSG], a[:, :, 0:MSG])

                # granule transpose, all on ONE HWDGE ring (sync):
                # g[q, c, gb, m] = f[m, c, gb*128 + q]
                g = tpool.tile([P, SC, GB, P], mybir.dt.bfloat16, tag="g")
                for c in range(SC):
                    nc.sync.dma_start(
                        out=g[:, c, :, :], in_=f[:, c, :], transpose=True
                    )
                # strided fp8 views: m stride = 2 bytes, off = byte offset
                g8 = g[:, :, :, :].bitcast(mybir.dt.float8e4).rearrange(
                    "q c b (m two) -> q c b two m", two=2
                )

                acc = ppool.tile([P, SC * NPAR], mybir.dt.float32, tag="acc")
                for c in range(SC):
                    for j in range(2 * GB):
                        gb, off = j // 2, j % 2
                        nc.tensor.matmul(
                            acc[:, c * NPAR : (c + 1) * NPAR],
                            g8[:, c, gb, off, :],
                            gsb8[:, j * NPAR : (j + 1) * NPAR],
                            start=(j == 0),
                            stop=(j == 2 * GB - 1),
                        )
                # exact-integer f32 -> i32 eviction on ACT
                c_i32 = cpool.tile([P, SC, NPAR], mybir.dt.int32, tag="c")
                nc.scalar.copy(
                    c_i32[:, :, :].rearrange("p c n -> p (c n)"), acc[:, :]
                )
                # mod 2 == AND 1 (bitVec op cannot cast, keep i32)
                e = epool.tile([P, SC, NPAR], mybir.dt.int32, tag="e")
                nc.vector.tensor_scalar(
                    e[:, :, :], c_i32[:, :, :], 1, None, mybir.AluOpType.bitwise_and
                )
                # parity into the f32 output-row tile
                nc.vector.tensor_copy(a[:, :, MSG:OUTW], e[:, :, :])

            def emit_store(si):
                # SWDGE plain f32 store from the idle gpsimd engine so stores
                # never head-of-line-block the sync ring (loads + transposes)
                a = a_tiles.pop(si)
                nc.gpsimd.dma_start(
                    out=out3[si, :, :, :].rearrange("c p k -> p c k"),
                    in_=a[:, :, 0:OUTW],
                )

            for it in range(n_super):
                emit_load(it)
            for it in range(n_super):
                emit_compute(it)
                emit_store(it)

    nc.compile()
    return nc


def prep_gp(Gp):
    """Pad Gp to 1024 rows, interleave-permute k, and emit fp8 byte slabs.

    Slab s = gb*2 + off (gb in 0..3, off in 0..1); row q of slab s holds
    Gp_pad[2*(128*gb + q) + off, :] as fp8e4 bytes (1.0 -> 0x38).
    """
    gp = np.asarray(Gp, dtype=np.float32)
    gp_pad = np.zeros((KPAD, NPAR), dtype=np.float32)
    gp_pad[:MSG] = gp
    b = np.where(gp_pad > 0.5, np.uint8(0x38), np.uint8(0)).astype(np.uint8)
    # b[k, n], k = 2*(128*gb + q) + off -> [gb, q, off, n] -> [q, (gb, off), n]
    slabs = b.reshape(GB, P, 2, NPAR).transpose(1, 0, 2, 3).reshape(P, 2 * GB * NPAR)
    return np.ascontiguousarray(slabs)


def kernel(message_bits, Gp):
    global LAST_RESULT
    msg = np.ascontiguousarray(np.asarray(message_bits, dtype=np.float32))
    assert msg.shape == (BATCH, MSG), msg.shape
    gsw = prep_gp(Gp)

    if "nc" not in _CACHE:
        _CACHE["nc"] = build_nc()
    nc = _CACHE["nc"]

    in_maps = [
        {"msg": msg[i * ROWS : (i + 1) * ROWS], "gp": gsw} for i in range(NCORES)
    ]
    res = run_bass_kernel_spmd(
        nc, in_maps, core_ids=list(range(NCORES)), trace=TRACE
    )
    LAST_RESULT = res
    return np.concatenate([r["out"] for r in res.results], axis=0)


# revision 4
# speedup vs baseline: 1.2905x; 1.0269x over previous
"""BCH/RS systematic encoder kernel for Trainium2 (8 NeuronCores, data parallel).

Computes out = concat([msg, (msg @ Gp) mod 2], axis=-1) for
msg [16384, 1000] f32 of 0/1 bits and Gp [1000, 256] f32 of 0/1 bits.

Design (per core, 2048 rows, 8 superchunks of 2x128):
  - HWDGE (sync) load msg chunk f32 into the ld/st row tile A[:, :, 0:1000]
  - ACT cast f32 -> fp8e4 (0/1 exact) into F; fp8 pairs live in 16-bit granules
  - HWDGE xbar transpose of the GRANULE tile (2-byte lanes): 4 blocks of
    [128,128] granules per chunk -> G[q, m] = (msg[m, 2K], msg[m, 2K+1]),
    K = 128*gb + q.  Half the xbar traffic of a bf16 transpose.
  - 8 accumulating fp8 matmuls per chunk read G with byte-strided APs
    (even/odd k split); the k-interleave is folded into the host-side Gp
    slab layout (permuting the contraction index is free).
  - ACT psum->i32, DVE AND 1 (mod 2), DVE i32->f32 into A[:, :, 1000:1256]
  - SWDGE (gpsimd) store of the full f32 rows [p, c, 1256]
HBM traffic/core = 8.19 MB read + 10.29 MB write (the minimum); xbar adds
only 2.1 MB each way on the shared SDMA fabric.
"""

import os
import sys

import numpy as np

if os.path.isdir("/opt/trn_rl_repo") and "/opt/trn_rl_repo" not in sys.path:
    sys.path.insert(0, "/opt/trn_rl_repo")

import ml_dtypes

import concourse.bacc as bacc
import concourse.mybir as mybir
import concourse.tile as tile
from concourse.bass_utils import run_bass_kernel_spmd

BATCH = 16384
MSG = 1000
NPAR = 256
NCORES = 8
ROWS = BATCH // NCORES  # 2048
P = 128
KPAD = 1024  # fp8 columns after pad
NGRAN = KPAD // 2  # 512 16-bit granules
GB = NGRAN // P  # 4 granule blocks per chunk
OUTW = MSG + NPAR  # 1256

# test.py pokes these for profiling
TRACE = False
LAST_RESULT = None

_CACHE = {}


def build_nc(rows=ROWS):
    """Emit the Bass/Tile IR for one core handling `rows` rows."""
    nc = bacc.Bacc("TRN2", target_bir_lowering=False, debug=False)
    msg = nc.dram_tensor("msg", [rows, MSG], mybir.dt.float32, kind="ExternalInput")
    gp = nc.dram_tensor("gp", [P, 2 * GB * NPAR], mybir.dt.uint8, kind="ExternalInput")
    out = nc.dram_tensor("out", [rows, OUTW], mybir.dt.float32, kind="ExternalOutput")

    SC = 2  # chunks per superchunk
    n_super = rows // (SC * P)
    msg3 = msg[:, :].rearrange("(s c p) k -> s c p k", c=SC, p=P)
    out3 = out[:, :].rearrange("(s c p) k -> s c p k", c=SC, p=P)

    with tile.TileContext(nc) as tc:
        with (
            tc.tile_pool(name="gpool", bufs=1) as gpool,
            tc.tile_pool(name="apool", bufs=min(n_super, 8)) as apool,
            tc.tile_pool(name="fpool", bufs=4) as fpool,
            tc.tile_pool(name="tpool", bufs=4) as tpool,
            tc.tile_pool(name="cpool", bufs=3) as cpool,
            tc.tile_pool(name="epool", bufs=3) as epool,
            tc.tile_pool(name="ppool", bufs=6, space="PSUM") as ppool,
        ):
            # Gp slabs resident in SBUF: row q of slab s=(gb,off) holds
            # Gp_pad[2*(128*gb+q)+off, :] as fp8 bytes
            gsb = gpool.tile([P, 2 * GB * NPAR], mybir.dt.uint8)
            nc.sync.dma_start(out=gsb[:, :], in_=gp[:, :])
            gsb8 = gsb[:, :].bitcast(mybir.dt.float8e4)

            a_tiles = {}

            def emit_load(si):
                # full output row in f32: cols 0:1000 msg, 1000:1256 parity
                a = apool.tile([P, SC, OUTW], mybir.dt.float32, tag="a")
                nc.sync.dma_start(
                    out=a[:, :, 0:MSG], in_=msg3[si, :, :, :].rearrange("c p k -> p c k")
                )
                a_tiles[si] = a

            def emit_compute(si):
                a = a_tiles[si]
                # fp8 cast target, allocated as bf16 so the xbar sees 2-byte
                # granules; each granule = (fp8 k=2j, fp8 k=2j+1)
                f = fpool.tile([P, SC, NGRAN], mybir.dt.bfloat16, tag="f")
                f8 = f[:, :, :].bitcast(mybir.dt.float8e4)  # [P, SC, KPAD]
                # zero the pad so pad-row garbage can't turn into NaN*0 in PSUM
                nc.vector.memset(f[:, :, MSG // 2 :], 0)
                nc.scalar.copy(f8[:, :, 0:MSG], a[:, :, 0:MSG])

                # ONE granule-transpose call per superchunk (dispatch cost is
                # per call): g[q, c*GB+gb, m] = f[m, c, gb*128 + q]
                g = tpool.tile([P, SC * GB, P], mybir.dt.bfloat16, tag="g")
                nc.sync.dma_start(
                    out=g[:, :, :],
                    in_=f[:, :, :].rearrange("p c g -> p (c g)"),
                    transpose=True,
                )
                # strided fp8 views: m stride = 2 bytes, off = byte offset
                g8 = g[:, :, :].bitcast(mybir.dt.float8e4).rearrange(
                    "q b (m two) -> q b two m", two=2
                )

                acc = ppool.tile([P, SC * NPAR], mybir.dt.float32, tag="acc")
                for c in range(SC):
                    for j in range(2 * GB):
                        gb, off = j // 2, j % 2
                        nc.tensor.matmul(
                            acc[:, c * NPAR : (c + 1) * NPAR],
                            g8[:, c * GB + gb, off, :],
                            gsb8[:, j * NPAR : (j + 1) * NPAR],
                            start=(j == 0),
                            stop=(j == 2 * GB - 1),
                        )
                # psum eviction + mod2 all on DVE so ACT only does casts:
                # f32 -> i32 (exact), AND 1, i32 -> f32 into the row tile
                c_i32 = cpool.tile([P, SC, NPAR], mybir.dt.int32, tag="c")
                nc.vector.tensor_copy(
                    c_i32[:, :, :].rearrange("p c n -> p (c n)"), acc[:, :]
                )
                e = epool.tile([P, SC, NPAR], mybir.dt.int32, tag="e")
                nc.vector.tensor_scalar(
                    e[:, :, :], c_i32[:, :, :], 1, None, mybir.AluOpType.bitwise_and
                )
                # parity into the f32 output-row tile
                nc.vector.tensor_copy(a[:, :, MSG:OUTW], e[:, :, :])

            def emit_store(si):
                # SWDGE plain f32 store from the idle gpsimd engine so stores
                # never head-of-line-block the sync ring (loads + transposes)
                a = a_tiles.pop(si)
                nc.gpsimd.dma_start(
                    out=out3[si, :, :, :].rearrange("c p k -> p c k"),
                    in_=a[:, :, 0:OUTW],
                )

            # interleaved emission: transposes dispatch on the sync ring with
            # at most LOOKAHEAD loads queued ahead of them in the FIFO
            LOOKAHEAD = 3
            for it in range(min(LOOKAHEAD, n_super)):
                emit_load(it)
            for it in range(n_super):
                emit_compute(it)
                emit_store(it)
                if it + LOOKAHEAD < n_super:
                    emit_load(it + LOOKAHEAD)

    nc.compile()
    return nc


def prep_gp(Gp):
    """Pad Gp to 1024 rows, interleave-permute k, and emit fp8 byte slabs.

    Slab s = gb*2 + off (gb in 0..3, off in 0..1); row q of slab s holds
    Gp_pad[2*(128*gb + q) + off, :] as fp8e4 bytes (1.0 -> 0x38).
    """
    gp = np.asarray(Gp, dtype=np.float32)
    gp_pad = np.zeros((KPAD, NPAR), dtype=np.float32)
    gp_pad[:MSG] = gp
    b = np.where(gp_pad > 0.5, np.uint8(0x38), np.uint8(0)).astype(np.uint8)
    # b[k, n], k = 2*(128*gb + q) + off -> [gb, q, off, n] -> [q, (gb, off), n]
    slabs = b.reshape(GB, P, 2, NPAR).transpose(1, 0, 2, 3).reshape(P, 2 * GB * NPAR)
    return np.ascontiguousarray(slabs)


def kernel(message_bits, Gp):
    global LAST_RESULT
    msg = np.ascontiguousarray(np.asarray(message_bits, dtype=np.float32))
    assert msg.shape == (BATCH, MSG), msg.shape
    gsw = prep_gp(Gp)

    if "nc" not in _CACHE:
        _CACHE["nc"] = build_nc()
    nc = _CACHE["nc"]

    in_maps = [
        {"msg": msg[i * ROWS : (i + 1) * ROWS], "gp": gsw} for i in range(NCORES)
    ]
    res = run_bass_kernel_spmd(
        nc, in_maps, core_ids=list(range(NCORES)), trace=TRACE
    )
    LAST_RESULT = res
    return np.concatenate([r["out"] for r in res.results], axis=0)


# revision 8
# speedup vs baseline: 2.1632x; 1.6763x over previous
"""BCH/RS systematic encoder kernel for Trainium2 (8 NeuronCores, data parallel).

Computes out = concat([msg, (msg @ Gp) mod 2], axis=-1) for
msg [16384, 1000] f32 of 0/1 bits and Gp [1000, 256] f32 of 0/1 bits.

Design (per core, 2048 rows, 8 superchunks of 2x128):
  - HWDGE (sync) load msg chunk f32 into the ld/st row tile A[:, :, 0:1000]
  - ACT cast f32 -> fp8e4 (0/1 exact) into F; fp8 pairs live in 16-bit granules
  - HWDGE xbar transpose of the GRANULE tile (2-byte lanes): 4 blocks of
    [128,128] granules per chunk -> G[q, m] = (msg[m, 2K], msg[m, 2K+1]),
    K = 128*gb + q.  Half the xbar traffic of a bf16 transpose.
  - 8 accumulating fp8 matmuls per chunk read G with byte-strided APs
    (even/odd k split); the k-interleave is folded into the host-side Gp
    slab layout (permuting the contraction index is free).
  - ACT psum->i32, DVE AND 1 (mod 2), DVE i32->f32 into A[:, :, 1000:1256]
  - SWDGE (gpsimd) store of the full f32 rows [p, c, 1256]
HBM traffic/core = 8.19 MB read + 10.29 MB write (the minimum); xbar adds
only 2.1 MB each way on the shared SDMA fabric.
"""

import os
import sys

import numpy as np

if os.path.isdir("/opt/trn_rl_repo") and "/opt/trn_rl_repo" not in sys.path:
    sys.path.insert(0, "/opt/trn_rl_repo")

import ml_dtypes

import concourse.bacc as bacc
import concourse.mybir as mybir
import concourse.tile as tile
from concourse.bass_utils import run_bass_kernel_spmd
from concourse.masks import make_identity

BATCH = 16384
MSG = 1000
NPAR = 256
NCORES = 8
ROWS = BATCH // NCORES  # 2048
P = 128
KPAD = 1024  # fp8 columns after pad
NGRAN = KPAD // 2  # 512 16-bit granules
GB = NGRAN // P  # 4 granule blocks per chunk
OUTW = MSG + NPAR  # 1256

# test.py pokes these for profiling
TRACE = False
LAST_RESULT = None

_CACHE = {}


def build_nc(rows=ROWS):
    """Emit the Bass/Tile IR for one core handling `rows` rows."""
    nc = bacc.Bacc("TRN2", target_bir_lowering=False, debug=False)
    msg = nc.dram_tensor("msg", [rows, MSG], mybir.dt.float32, kind="ExternalInput")
    gp = nc.dram_tensor("gp", [P, 2 * GB * NPAR], mybir.dt.uint8, kind="ExternalInput")
    out = nc.dram_tensor("out", [rows, OUTW], mybir.dt.float32, kind="ExternalOutput")

    SC = 2  # chunks per superchunk
    n_super = rows // (SC * P)
    msg3 = msg[:, :].rearrange("(s c p) k -> s c p k", c=SC, p=P)
    out3 = out[:, :].rearrange("(s c p) k -> s c p k", c=SC, p=P)

    with tile.TileContext(nc) as tc:
        with (
            tc.tile_pool(name="gpool", bufs=1) as gpool,
            tc.tile_pool(name="apool", bufs=min(n_super, 8)) as apool,
            tc.tile_pool(name="fpool", bufs=4) as fpool,
            tc.tile_pool(name="tpool", bufs=4) as tpool,
            tc.tile_pool(name="cpool", bufs=3) as cpool,
            tc.tile_pool(name="epool", bufs=3) as epool,
            tc.tile_pool(name="ppool", bufs=3, space="PSUM") as ppool,
            tc.tile_pool(name="tpsum", bufs=4, space="PSUM") as tpsum,
        ):
            # Gp slabs resident in SBUF: row q of slab s=(gb,off) holds
            # Gp_pad[2*(128*gb+q)+off, :] as fp8 bytes
            gsb = gpool.tile([P, 2 * GB * NPAR], mybir.dt.uint8)
            nc.sync.dma_start(out=gsb[:, :], in_=gp[:, :])
            gsb8 = gsb[:, :].bitcast(mybir.dt.float8e4)
            # identity for PE transpose-mode (raw granule movement)
            ident = gpool.tile([P, P], mybir.dt.bfloat16)
            make_identity(nc, ident[:, :])

            a_tiles = {}

            def emit_load(si):
                # full output row in f32: cols 0:1000 msg, 1000:1256 parity
                a = apool.tile([P, SC, OUTW], mybir.dt.float32, tag="a")
                nc.sync.dma_start(
                    out=a[:, :, 0:MSG], in_=msg3[si, :, :, :].rearrange("c p k -> p c k")
                )
                a_tiles[si] = a

            def emit_compute(si):
                a = a_tiles[si]
                # fp8 cast target, allocated as bf16 so the xbar sees 2-byte
                # granules; each granule = (fp8 k=2j, fp8 k=2j+1)
                f = fpool.tile([P, SC, NGRAN], mybir.dt.bfloat16, tag="f")
                f8 = f[:, :, :].bitcast(mybir.dt.float8e4)  # [P, SC, KPAD]
                # zero the pad so pad-row garbage can't turn into NaN*0 in PSUM
                nc.vector.memset(f[:, :, MSG // 2 :], 0)
                nc.scalar.copy(f8[:, :, 0:MSG], a[:, :, 0:MSG])

                # PE transpose-mode on the 2-byte granules (raw movement, no
                # DMA -> no Tile xbar/copy serialization):
                # pt[q, gb, m] = f[m, c, gb*128 + q], then one DVE copy/chunk
                g = tpool.tile([P, SC * GB, P], mybir.dt.bfloat16, tag="g")
                # strided fp8 views: m stride = 2 bytes, off = byte offset
                g8 = g[:, :, :].bitcast(mybir.dt.float8e4).rearrange(
                    "q b (m two) -> q b two m", two=2
                )
                acc = ppool.tile([P, SC * NPAR], mybir.dt.float32, tag="acc")
                for c in range(SC):
                    pt = tpsum.tile([P, GB, P], mybir.dt.bfloat16, tag="pt")
                    for gb in range(GB):
                        nc.tensor.transpose(
                            pt[:, gb, :],
                            f[:, c, gb * P : (gb + 1) * P],
                            ident[:, :],
                        )
                    nc.vector.tensor_copy(
                        g[:, c * GB : (c + 1) * GB, :], pt[:, :, :]
                    )
                    for j in range(2 * GB):
                        gb, off = j // 2, j % 2
                        nc.tensor.matmul(
                            acc[:, c * NPAR : (c + 1) * NPAR],
                            g8[:, c * GB + gb, off, :],
                            gsb8[:, j * NPAR : (j + 1) * NPAR],
                            start=(j == 0),
                            stop=(j == 2 * GB - 1),
                        )
                # psum f32 -> i32 on ACT (exact), AND 1 + i32 -> f32 on DVE
                c_i32 = cpool.tile([P, SC, NPAR], mybir.dt.int32, tag="c")
                nc.scalar.copy(
                    c_i32[:, :, :].rearrange("p c n -> p (c n)"), acc[:, :]
                )
                e = epool.tile([P, SC, NPAR], mybir.dt.int32, tag="e")
                nc.vector.tensor_scalar(
                    e[:, :, :], c_i32[:, :, :], 1, None, mybir.AluOpType.bitwise_and
                )
                # parity into the f32 output-row tile
                nc.vector.tensor_copy(a[:, :, MSG:OUTW], e[:, :, :])

            def emit_store(si):
                # SWDGE plain f32 store from the idle gpsimd engine so stores
                # never head-of-line-block the sync ring (loads + transposes)
                a = a_tiles.pop(si)
                nc.gpsimd.dma_start(
                    out=out3[si, :, :, :].rearrange("c p k -> p c k"),
                    in_=a[:, :, 0:OUTW],
                )

            # all loads upfront: pure copies on the sync ring, FIFO drain
            # gives ordered incremental completion; stores ride the gpsimd
            # (SWDGE) ring so the two streams share SDMA bandwidth fairly
            for it in range(n_super):
                emit_load(it)
            for it in range(n_super):
                emit_compute(it)
                emit_store(it)

    nc.compile()
    return nc


def prep_gp(Gp):
    """Pad Gp to 1024 rows, interleave-permute k, and emit fp8 byte slabs.

    Slab s = gb*2 + off (gb in 0..3, off in 0..1); row q of slab s holds
    Gp_pad[2*(128*gb + q) + off, :] as fp8e4 bytes (1.0 -> 0x38).
    """
    gp = np.asarray(Gp, dtype=np.float32)
    gp_pad = np.zeros((KPAD, NPAR), dtype=np.float32)
    gp_pad[:MSG] = gp
    b = np.where(gp_pad > 0.5, np.uint8(0x38), np.uint8(0)).astype(np.uint8)
    # b[k, n], k = 2*(128*gb + q) + off -> [gb, q, off, n] -> [q, (gb, off), n]
    slabs = b.reshape(GB, P, 2, NPAR).transpose(1, 0, 2, 3).reshape(P, 2 * GB * NPAR)
    return np.ascontiguousarray(slabs)


def kernel(message_bits, Gp):
    global LAST_RESULT
    msg = np.ascontiguousarray(np.asarray(message_bits, dtype=np.float32))
    assert msg.shape == (BATCH, MSG), msg.shape
    gsw = prep_gp(Gp)

    if "nc" not in _CACHE:
        _CACHE["nc"] = build_nc()
    nc = _CACHE["nc"]

    in_maps = [
        {"msg": msg[i * ROWS : (i + 1) * ROWS], "gp": gsw} for i in range(NCORES)
    ]
    res = run_bass_kernel_spmd(
        nc, in_maps, core_ids=list(range(NCORES)), trace=TRACE
    )
    LAST_RESULT = res
    return np.concatenate([r["out"] for r in res.results], axis=0)
